# revision 27
# baseline (speedup 1.0000x reference)
"""Trainium2 Bass kernel for nn_DRAM_MAC_temporal_encoding (polynomial attention).

Math (QK_mul=1):
    out = sum_i coef_i * (x @ (y-OFF)^i) * decay
        = (x * decay) @ P(y-OFF)            # P = Horner cubic, elementwise
so the whole problem is ONE [S,64]@[64,S] matmul per (b,h) head plus the
output write -> memory-bound. The tiny elementwise prep (poly on y,
row-scaling x, transposes, fp16 cast) runs on host; the device does
matmuls + store.

QK_mul=0: out = sum_i coef_i * ((x*d^i) @ (y-OFF)^i)
        = concat_i(x*d^i) @ concat_i(coef_i*(y-OFF)^i)   # K=256, same kernel
          with 2 stacked K=128 chunks instead of 1 K=64 chunk.

Precision: the harness gate is rel_err < 2e-2; a single fp16 matmul
(fp32 PSUM accumulate) + fp16 output store lands ~2e-4, so no hi/lo
splitting is needed. fp16 output halves the dominant HBM store traffic
vs fp32 (the 16 per-core DMA engines cap out at ~379 GB/s aggregate).

Drain: PE can only write fp32 PSUM; every output element must cross
DVE (1.04 ns/elem from PSUM) or the Act engine (0.83 ns/elem) once.
DVE alone would be the bottleneck (~110 us/core), so each PSUM tile is
drained by both engines on disjoint column ranges.

Sharding: 24 (b,h) heads -> 3 per core across 8 cores.
"""

import ml_dtypes
import numpy as np

import concourse.mybir as mybir
import concourse.tile as tile
from concourse import bacc
from concourse.bass_utils import run_bass_kernel_spmd

C = [0.17393044, 0.15653739, 0.14088365, 0.12679529, 5.51975209,
     4.96777688, 4.4709992, -1.44776001, -1.30298401, 46.05483778]
MAX_ORDER = 3
X_MAX = 0.9
OFFSET = 0.45

B, H, S, D = 2, 12, 2048, 64
BH = B * H
N_CORES = 8
BLK = BH // N_CORES  # heads per core

M_TILE = 128   # output rows per matmul (PSUM partitions)
N_TILE = 512   # output cols per matmul (one fp32 PSUM bank)
FUSE = 2       # PSUM banks per ps tile (2 banks x 4 bufs = 8 banks);
# small tiles + deep bufs give WAR slack so semaphore latency between
# PE -> drain -> PE never gates the pipeline. Each ps tile is drained
# whole by ONE engine, alternating DVE/Act per tile, so no ps tile
# waits on two engines to be freed.
# Row-tiles per output store. DMA descriptors cost ~214 ns fixed +
# ~0.0245 ns/B (measured), so bigger contiguous per-partition segments
# win big: stores use a partition-major DRAM layout [128, S/128, S] so
# each partition line is G*4 KB contiguous (32 KB at G=8 -> ~32 GB/s per
# engine vs 13.5 at 4 KB). Host un-permutes while upcasting. The first
# head uses graded groups [1,1,2,4,8] so the DMA engines get fed as soon
# as the first row-tile drains.
G_ST = 8

F16 = np.float16

_NC_CACHE = {}


def _coefs():
    cs = []
    idx = 0
    for i in range(MAX_ORDER + 1):
        n_j = MAX_ORDER - i + 1
        cs.append(sum(C[idx + j] * X_MAX ** j for j in range(n_j)))
        idx += n_j
    return cs  # [c0, c1, c2, c3]


def _build_nc(kcs):
    """Device kernel: per core, BLK independent [S,S] output blocks.

    kcs=1 (QK path): K=64. FWL (overlapped weight load) needs
    NumWeights==128, else a serial 148ns LDWEIGHTS gates every matmul,
    so lhsT is [real 64 rows; 64 zero rows] (zeros memset on gpsimd, not
    DMAed). Since weight rows 64-127 are zero, rhs rows 64-127 are
    don't-cares: w tiles pack TWO heads per [128,S] tile with no padding
    (head0 reads [P0;P1] against [at0;0], head1 against [0;at1]).
    kcs=2 (no-QK path): two stacked K=128 chunks accumulated per bank.
    """
    kp = 2 * D
    nc = bacc.Bacc(None, target_bir_lowering=False)
    if kcs == 1:
        at_d = nc.dram_tensor("at", [BLK, D, S], mybir.dt.float16,
                              kind="ExternalInput")
        # w packed per head pair; BLK=3 -> 2 tiles, second half-filled
        w_d = nc.dram_tensor("w", [(BLK + 1) // 2, 2 * D, S],
                             mybir.dt.float16, kind="ExternalInput")
    else:
        at_d = nc.dram_tensor("at", [BLK, kcs, kp, S], mybir.dt.float16,
                              kind="ExternalInput")
        w_d = nc.dram_tensor("w", [BLK, kcs, kp, S], mybir.dt.float16,
                             kind="ExternalInput")
    # partition-major: out_p[blk, p, st, c] = out[blk, st*128 + p, c]
    out_d = nc.dram_tensor("out", [BLK, M_TILE, S // M_TILE, S],
                           mybir.dt.float16, kind="ExternalOutput")

    with tile.TileContext(nc) as tc:
        with (
            tc.tile_pool(name="inp", bufs=1) as inp,
            tc.tile_pool(name="ps", bufs=4, space="PSUM") as psp,
            tc.tile_pool(name="outp", bufs=3) as outp,
            tc.tile_pool(name="outw", bufs=2) as outw,
        ):
            # Prefetch every input tile up front so the steady-state DMA
            # queues carry only output stores.
            at_ts, w_ts = {}, {}
            if kcs == 1:
                for blk in range(BLK):
                    ta = inp.tile([kp, S], mybir.dt.float16, tag=f"at{blk}")
                    at_ts[(blk, 0)] = ta
                for pair in range((BLK + 1) // 2):
                    tw = inp.tile([kp, S], mybir.dt.float16, tag=f"w{pair}")
                    w_ts[(2 * pair, 0)] = tw
                    if 2 * pair + 1 < BLK:
                        w_ts[(2 * pair + 1, 0)] = tw
                # Zero the unused lhsT half of each at tile. Spread across
                # engines (they're all idle at startup) — serializing on
                # gpsimd (1.8us each) would gate the first matmul via the
                # monotonic cross-engine semaphores.
                def z(blk):
                    lo = (blk % 2) * D
                    ta = at_ts[(blk, 0)]
                    return ta[D - lo:2 * D - lo]

                # (gpsimd memset is software-DGE = DMA-backed; keep these
                # on the compute engines so they cost no DMA bandwidth)
                nc.vector.memset(z(0), 0)
                nc.vector.memset(z(1), 0)
                nc.scalar.memzero(z(2))
                if BLK % 2:  # unpaired head's w bottom half is read (x0)
                    nc.scalar.memzero(w_ts[(BLK - 1, 0)][D:])  # keep finite
                # DMA issue order: first-needed chunks first (w pair0 +
                # at0 cols 0-1023 cover the first ~8 row-tiles).
                half_s = S // 2
                lo0 = 0
                nc.sync.dma_start(w_ts[(0, 0)][:, :half_s],
                                  w_d[0][:, :half_s])
                nc.sync.dma_start(at_ts[(0, 0)][lo0:lo0 + D, :half_s],
                                  at_d[0][:, :half_s])
                nc.sync.dma_start(w_ts[(0, 0)][:, half_s:],
                                  w_d[0][:, half_s:])
                nc.sync.dma_start(at_ts[(0, 0)][lo0:lo0 + D, half_s:],
                                  at_d[0][:, half_s:])
                for blk in range(1, BLK):
                    lo = (blk % 2) * D
                    nc.sync.dma_start(at_ts[(blk, 0)][lo:lo + D], at_d[blk])
                    if blk % 2 == 0 and blk // 2 < (BLK + 1) // 2:
                        pair = blk // 2
                        if 2 * pair + 1 < BLK:
                            nc.sync.dma_start(w_ts[(blk, 0)][:], w_d[pair])
                        else:
                            nc.sync.dma_start(w_ts[(blk, 0)][:D],
                                              w_d[pair, :D])
            else:
                for blk in range(BLK):
                    for c in range(kcs):
                        ta = inp.tile([kp, S], mybir.dt.float16,
                                      tag=f"at{blk}_{c}")
                        nc.sync.dma_start(ta[:], at_d[blk, c])
                        at_ts[(blk, c)] = ta
                        tw = inp.tile([kp, S], mybir.dt.float16,
                                      tag=f"w{blk}_{c}")
                        nc.sync.dma_start(tw[:], w_d[blk, c])
                        w_ts[(blk, c)] = tw
            groups = [1, 1, 2, 4, G_ST]
            n_full = S // M_TILE // G_ST
            for blk in range(BLK):
                g0 = 0
                for ng in (groups if blk == 0 else [G_ST] * n_full):
                    pool = outp if ng == G_ST else outw
                    ot = pool.tile([M_TILE, ng * S], mybir.dt.float16,
                                   tag=f"ot{ng}")
                    for gi in range(ng):
                        st = g0 + gi
                        for half in range(S // (FUSE * N_TILE)):
                            ps = psp.tile([M_TILE, FUSE * N_TILE],
                                          mybir.dt.float32, tag="ps")
                            for f in range(FUSE):
                                nt = half * FUSE + f
                                for c in range(kcs):
                                    nc.tensor.matmul(
                                        ps[:, f * N_TILE:(f + 1) * N_TILE],
                                        at_ts[(blk, c)][
                                            :,
                                            st * M_TILE:(st + 1) * M_TILE],
                                        w_ts[(blk, c)][
                                            :,
                                            nt * N_TILE:(nt + 1) * N_TILE],
                                        start=(c == 0),
                                        stop=(c == kcs - 1),
                                    )
                            o = gi * S + half * FUSE * N_TILE
                            eng = nc.vector.tensor_copy if half == 0 \
                                else nc.scalar.copy
                            eng(ot[:, o:o + FUSE * N_TILE], ps[:])
                    nc.sync.dma_start(
                        out_d[blk, :, g0:g0 + ng, :], ot[:])
                    g0 += ng
    nc.compile()
    return nc


def _get_nc(kcs):
    if kcs not in _NC_CACHE:
        _NC_CACHE[kcs] = _build_nc(kcs)
    return _NC_CACHE[kcs]


def _prepare(x, y, dm, qk):
    """Host prep -> (at, w) fp16 arrays [BH, kcs, KP, S]."""
    c0, c1, c2, c3 = _coefs()
    yo = y - OFFSET  # [B,H,D,S]
    if qk:
        kcs = 1
        # at: just the real 64 lhsT rows per head; device zero-pads.
        at = np.ascontiguousarray(
            (x * dm[None, None, :, :]).transpose(0, 1, 3, 2)
            .reshape(BH, D, S), dtype=F16)
        # w: per core, heads packed in pairs [P_even; P_odd]; the last
        # (unpaired) head's bottom half is memset on device.
        P = (((c3 * yo + c2) * yo + c1) * yo + c0).reshape(BH, D, S)
        npair = (BLK + 1) // 2
        w = np.zeros((N_CORES, npair, 2 * D, S), dtype=F16)
        for c in range(N_CORES):
            for pair in range(npair):
                h = c * BLK + 2 * pair
                w[c, pair, :D] = P[h]
                if 2 * pair + 1 < BLK:
                    w[c, pair, D:] = P[h + 1]
    else:
        kcs, kp = 2, 2 * D
        at = np.empty((BH, kcs, kp, S), dtype=F16)
        w = np.empty((BH, kcs, kp, S), dtype=F16)
        d = dm[:, 0]
        di = np.ones_like(d)
        yi = np.ones_like(yo)
        for i, ci in enumerate((c0, c1, c2, c3)):
            c, half = divmod(i, 2)
            sl = slice(half * D, (half + 1) * D)
            at[:, c, sl] = (x * di[None, None, :, None]) \
                .transpose(0, 1, 3, 2).reshape(BH, D, S)
            w[:, c, sl] = (ci * yi).reshape(BH, D, S)
            di = di * d
            yi = yi * yo
    return at, w, kcs


def _in_maps(at, w):
    if at.ndim == 3:  # qk=1: at [BH,D,S], w [N_CORES,npair,2D,S]
        return [
            {"at": at[c * BLK:(c + 1) * BLK], "w": w[c]}
            for c in range(N_CORES)
        ]
    return [
        {"at": at[c * BLK:(c + 1) * BLK], "w": w[c * BLK:(c + 1) * BLK]}
        for c in range(N_CORES)
    ]


def kernel(**inputs):
    x = np.asarray(inputs["x"], dtype=np.float32)
    y = np.asarray(inputs["y"], dtype=np.float32)
    dm = np.asarray(inputs["decay_mask"], dtype=np.float32)
    qk = int(np.asarray(inputs["QK_mul"]))

    at, w, kcs = _prepare(x, y, dm, qk)
    nc = _get_nc(kcs)
    in_maps = _in_maps(at, w)
    res = None
    for attempt in range(3):
        try:
            res = run_bass_kernel_spmd(nc, in_maps,
                                       core_ids=list(range(N_CORES)))
            break
        except Exception:
            # transient NRT_EXEC_UNIT_UNRECOVERABLE wedges occur on busy axon
            # terminals; they clear after a pause
            if attempt == 2:
                raise
            import time
            time.sleep(45)

    out = np.empty((BH, S, S), dtype=np.float32)
    for c in range(N_CORES):
        # un-permute [BLK, 128, S/128, S] -> [BLK, S, S] while upcasting
        op = res.results[c]["out"]
        out[c * BLK:(c + 1) * BLK] = (
            op.transpose(0, 2, 1, 3).reshape(BLK, S, S))
    return out.reshape(B, H, S, S)


# revision 29
# speedup vs baseline: 1.0587x; 1.0587x over previous
"""Trainium2 Bass kernel for nn_DRAM_MAC_temporal_encoding (polynomial attention).

Math (QK_mul=1):
    out = sum_i coef_i * (x @ (y-OFF)^i) * decay
        = (x * decay) @ P(y-OFF)            # P = Horner cubic, elementwise
so the whole problem is ONE [S,64]@[64,S] matmul per (b,h) head plus the
output write -> memory-bound. The tiny elementwise prep (poly on y,
row-scaling x, transposes, fp16 cast) runs on host; the device does
matmuls + store.

QK_mul=0: out = sum_i coef_i * ((x*d^i) @ (y-OFF)^i)
        = concat_i(x*d^i) @ concat_i(coef_i*(y-OFF)^i)   # K=256, same kernel
          with 2 stacked K=128 chunks instead of 1 K=64 chunk.

Precision: the harness gate is rel_err < 2e-2; a single fp16 matmul
(fp32 PSUM accumulate) + fp16 output store lands ~2e-4, so no hi/lo
splitting is needed. fp16 output halves the dominant HBM store traffic
vs fp32 (the 16 per-core DMA engines cap out at ~379 GB/s aggregate).

Drain: PE can only write fp32 PSUM; every output element must cross
DVE (1.04 ns/elem from PSUM) or the Act engine (0.83 ns/elem) once.
DVE alone would be the bottleneck (~110 us/core), so each PSUM tile is
drained by both engines on disjoint column ranges.

Sharding: 24 (b,h) heads -> 3 per core across 8 cores.
"""

import ml_dtypes
import numpy as np

import concourse.mybir as mybir
import concourse.tile as tile
from concourse import bacc
from concourse.bass_utils import run_bass_kernel_spmd

C = [0.17393044, 0.15653739, 0.14088365, 0.12679529, 5.51975209,
     4.96777688, 4.4709992, -1.44776001, -1.30298401, 46.05483778]
MAX_ORDER = 3
X_MAX = 0.9
OFFSET = 0.45

B, H, S, D = 2, 12, 2048, 64
BH = B * H
N_CORES = 8
BLK = BH // N_CORES  # heads per core

M_TILE = 128   # output rows per matmul (PSUM partitions)
N_TILE = 512   # output cols per matmul (one fp32 PSUM bank)
FUSE = 2       # PSUM banks per ps tile (2 banks x 4 bufs = 8 banks);
# small tiles + deep bufs give WAR slack so semaphore latency between
# PE -> drain -> PE never gates the pipeline. Each ps tile is drained
# whole by ONE engine, alternating DVE/Act per tile, so no ps tile
# waits on two engines to be freed.
# Row-tiles per output store. Stores use a partition-major DRAM layout
# [128, S/128, S] so each partition line is G*4 KB contiguous; 16 KB
# descriptors stream at ~26.6 GB/s/engine (4 KB only ~25.4). G=8/32KB
# measured WORSE overall: same streaming rate but burstier feed and a
# 2x longer final-store tail. The first head uses graded groups
# [1,1,2,4,...] so the DMA engines get fed as soon as the first
# row-tile drains.
G_ST = 4

F16 = np.float16

_NC_CACHE = {}


def _coefs():
    cs = []
    idx = 0
    for i in range(MAX_ORDER + 1):
        n_j = MAX_ORDER - i + 1
        cs.append(sum(C[idx + j] * X_MAX ** j for j in range(n_j)))
        idx += n_j
    return cs  # [c0, c1, c2, c3]


def _build_nc(kcs):
    """Device kernel: per core, BLK independent [S,S] output blocks.

    kcs=1 (QK path): K=64. FWL (overlapped weight load) needs
    NumWeights==128, else a serial 148ns LDWEIGHTS gates every matmul,
    so lhsT is [real 64 rows; 64 zero rows] (zeros memset on gpsimd, not
    DMAed). Since weight rows 64-127 are zero, rhs rows 64-127 are
    don't-cares: w tiles pack TWO heads per [128,S] tile with no padding
    (head0 reads [P0;P1] against [at0;0], head1 against [0;at1]).
    kcs=2 (no-QK path): two stacked K=128 chunks accumulated per bank.
    """
    kp = 2 * D
    nc = bacc.Bacc(None, target_bir_lowering=False)
    if kcs == 1:
        at_d = nc.dram_tensor("at", [BLK, D, S], mybir.dt.float16,
                              kind="ExternalInput")
        # w packed per head pair; BLK=3 -> 2 tiles, second half-filled
        w_d = nc.dram_tensor("w", [(BLK + 1) // 2, 2 * D, S],
                             mybir.dt.float16, kind="ExternalInput")
    else:
        at_d = nc.dram_tensor("at", [BLK, kcs, kp, S], mybir.dt.float16,
                              kind="ExternalInput")
        w_d = nc.dram_tensor("w", [BLK, kcs, kp, S], mybir.dt.float16,
                             kind="ExternalInput")
    # partition-major: out_p[blk, p, st, c] = out[blk, st*128 + p, c]
    out_d = nc.dram_tensor("out", [BLK, M_TILE, S // M_TILE, S],
                           mybir.dt.float16, kind="ExternalOutput")

    with tile.TileContext(nc) as tc:
        with (
            tc.tile_pool(name="inp", bufs=1) as inp,
            tc.tile_pool(name="ps", bufs=4, space="PSUM") as psp,
            tc.tile_pool(name="outp", bufs=3) as outp,
            tc.tile_pool(name="outw", bufs=2) as outw,
        ):
            # Prefetch every input tile up front so the steady-state DMA
            # queues carry only output stores.
            at_ts, w_ts = {}, {}
            if kcs == 1:
                for blk in range(BLK):
                    ta = inp.tile([kp, S], mybir.dt.float16, tag=f"at{blk}")
                    at_ts[(blk, 0)] = ta
                for pair in range((BLK + 1) // 2):
                    tw = inp.tile([kp, S], mybir.dt.float16, tag=f"w{pair}")
                    w_ts[(2 * pair, 0)] = tw
                    if 2 * pair + 1 < BLK:
                        w_ts[(2 * pair + 1, 0)] = tw
                # Zero the unused lhsT half of each at tile. Spread across
                # engines (they're all idle at startup) — serializing on
                # gpsimd (1.8us each) would gate the first matmul via the
                # monotonic cross-engine semaphores.
                def z(blk):
                    lo = (blk % 2) * D
                    ta = at_ts[(blk, 0)]
                    return ta[D - lo:2 * D - lo]

                # (gpsimd memset is software-DGE = DMA-backed; keep these
                # on the compute engines so they cost no DMA bandwidth)
                nc.vector.memset(z(0), 0)
                nc.vector.memset(z(1), 0)
                nc.scalar.memzero(z(2))
                if BLK % 2:  # unpaired head's w bottom half is read (x0)
                    nc.scalar.memzero(w_ts[(BLK - 1, 0)][D:])  # keep finite
                # DMA issue order: first-needed chunks first (w pair0 +
                # at0 cols 0-1023 cover the first ~8 row-tiles).
                half_s = S // 2
                lo0 = 0
                nc.sync.dma_start(w_ts[(0, 0)][:, :half_s],
                                  w_d[0][:, :half_s])
                nc.sync.dma_start(at_ts[(0, 0)][lo0:lo0 + D, :half_s],
                                  at_d[0][:, :half_s])
                nc.sync.dma_start(w_ts[(0, 0)][:, half_s:],
                                  w_d[0][:, half_s:])
                nc.sync.dma_start(at_ts[(0, 0)][lo0:lo0 + D, half_s:],
                                  at_d[0][:, half_s:])
                for blk in range(1, BLK):
                    lo = (blk % 2) * D
                    nc.sync.dma_start(at_ts[(blk, 0)][lo:lo + D], at_d[blk])
                    if blk % 2 == 0 and blk // 2 < (BLK + 1) // 2:
                        pair = blk // 2
                        if 2 * pair + 1 < BLK:
                            nc.sync.dma_start(w_ts[(blk, 0)][:], w_d[pair])
                        else:
                            nc.sync.dma_start(w_ts[(blk, 0)][:D],
                                              w_d[pair, :D])
            else:
                for blk in range(BLK):
                    for c in range(kcs):
                        ta = inp.tile([kp, S], mybir.dt.float16,
                                      tag=f"at{blk}_{c}")
                        nc.sync.dma_start(ta[:], at_d[blk, c])
                        at_ts[(blk, c)] = ta
                        tw = inp.tile([kp, S], mybir.dt.float16,
                                      tag=f"w{blk}_{c}")
                        nc.sync.dma_start(tw[:], w_d[blk, c])
                        w_ts[(blk, c)] = tw
            groups = [1, 1, 2, 4] + [G_ST] * ((S // M_TILE - 8) // G_ST)
            assert sum(groups) == S // M_TILE
            n_full = S // M_TILE // G_ST
            for blk in range(BLK):
                g0 = 0
                for ng in (groups if blk == 0 else [G_ST] * n_full):
                    pool = outp if ng == G_ST else outw
                    ot = pool.tile([M_TILE, ng * S], mybir.dt.float16,
                                   tag=f"ot{ng}")
                    for gi in range(ng):
                        st = g0 + gi
                        for half in range(S // (FUSE * N_TILE)):
                            ps = psp.tile([M_TILE, FUSE * N_TILE],
                                          mybir.dt.float32, tag="ps")
                            for f in range(FUSE):
                                nt = half * FUSE + f
                                for c in range(kcs):
                                    nc.tensor.matmul(
                                        ps[:, f * N_TILE:(f + 1) * N_TILE],
                                        at_ts[(blk, c)][
                                            :,
                                            st * M_TILE:(st + 1) * M_TILE],
                                        w_ts[(blk, c)][
                                            :,
                                            nt * N_TILE:(nt + 1) * N_TILE],
                                        start=(c == 0),
                                        stop=(c == kcs - 1),
                                    )
                            o = gi * S + half * FUSE * N_TILE
                            eng = nc.vector.tensor_copy if half == 0 \
                                else nc.scalar.copy
                            eng(ot[:, o:o + FUSE * N_TILE], ps[:])
                    nc.sync.dma_start(
                        out_d[blk, :, g0:g0 + ng, :], ot[:])
                    g0 += ng
    nc.compile()
    return nc


def _get_nc(kcs):
    if kcs not in _NC_CACHE:
        _NC_CACHE[kcs] = _build_nc(kcs)
    return _NC_CACHE[kcs]


def _prepare(x, y, dm, qk):
    """Host prep -> (at, w) fp16 arrays [BH, kcs, KP, S]."""
    c0, c1, c2, c3 = _coefs()
    yo = y - OFFSET  # [B,H,D,S]
    if qk:
        kcs = 1
        # at: just the real 64 lhsT rows per head; device zero-pads.
        at = np.ascontiguousarray(
            (x * dm[None, None, :, :]).transpose(0, 1, 3, 2)
            .reshape(BH, D, S), dtype=F16)
        # w: per core, heads packed in pairs [P_even; P_odd]; the last
        # (unpaired) head's bottom half is memset on device.
        P = (((c3 * yo + c2) * yo + c1) * yo + c0).reshape(BH, D, S)
        npair = (BLK + 1) // 2
        w = np.zeros((N_CORES, npair, 2 * D, S), dtype=F16)
        for c in range(N_CORES):
            for pair in range(npair):
                h = c * BLK + 2 * pair
                w[c, pair, :D] = P[h]
                if 2 * pair + 1 < BLK:
                    w[c, pair, D:] = P[h + 1]
    else:
        kcs, kp = 2, 2 * D
        at = np.empty((BH, kcs, kp, S), dtype=F16)
        w = np.empty((BH, kcs, kp, S), dtype=F16)
        d = dm[:, 0]
        di = np.ones_like(d)
        yi = np.ones_like(yo)
        for i, ci in enumerate((c0, c1, c2, c3)):
            c, half = divmod(i, 2)
            sl = slice(half * D, (half + 1) * D)
            at[:, c, sl] = (x * di[None, None, :, None]) \
                .transpose(0, 1, 3, 2).reshape(BH, D, S)
            w[:, c, sl] = (ci * yi).reshape(BH, D, S)
            di = di * d
            yi = yi * yo
    return at, w, kcs


def _in_maps(at, w):
    if at.ndim == 3:  # qk=1: at [BH,D,S], w [N_CORES,npair,2D,S]
        return [
            {"at": at[c * BLK:(c + 1) * BLK], "w": w[c]}
            for c in range(N_CORES)
        ]
    return [
        {"at": at[c * BLK:(c + 1) * BLK], "w": w[c * BLK:(c + 1) * BLK]}
        for c in range(N_CORES)
    ]


def kernel(**inputs):
    x = np.asarray(inputs["x"], dtype=np.float32)
    y = np.asarray(inputs["y"], dtype=np.float32)
    dm = np.asarray(inputs["decay_mask"], dtype=np.float32)
    qk = int(np.asarray(inputs["QK_mul"]))

    at, w, kcs = _prepare(x, y, dm, qk)
    nc = _get_nc(kcs)
    in_maps = _in_maps(at, w)
    res = None
    for attempt in range(3):
        try:
            res = run_bass_kernel_spmd(nc, in_maps,
                                       core_ids=list(range(N_CORES)))
            break
        except Exception:
            # transient NRT_EXEC_UNIT_UNRECOVERABLE wedges occur on busy axon
            # terminals; they clear after a pause
            if attempt == 2:
                raise
            import time
            time.sleep(45)

    out = np.empty((BH, S, S), dtype=np.float32)
    for c in range(N_CORES):
        # un-permute [BLK, 128, S/128, S] -> [BLK, S, S] while upcasting
        op = res.results[c]["out"]
        out[c * BLK:(c + 1) * BLK] = (
            op.transpose(0, 2, 1, 3).reshape(BLK, S, S))
    return out.reshape(B, H, S, S)
